# revision 25
# baseline (speedup 1.0000x reference)
"""EuclideanCodebook (VQ) kernel for 8 Trainium2 NeuronCores.

Computes, for x [16, 4096, 256] f32 and embed [2048, 256] f32:
    dist[n, k] = -(|x_n|^2 - 2 x_n.e_k + |e_k|^2)
    ind[n]     = argmax_k dist[n, k]
    quantize   = embed[ind]
returning (quantize [16,4096,256] f32, ind [16,4096] int32).

Strategy: data-parallel over the flattened token axis (65536 tokens ->
8192 per core).  argmax_k dist = argmax_k (x.e_k - 0.5|e_k|^2).

Exact-f32 scores at 3 cycles/row instead of the hardware fp32 matmul's
4: the PE's fp32r mode rounds inputs to 11 mantissa bits, so split
x = hi + lo and W = Whi + Wlo (host-side truncation at bit 11; every
term is fp32r-exact) and accumulate hi.Whi + hi.Wlo + lo.Whi in PSUM
(the dropped lo.Wlo term is ~2^-22 relative, below fp32 matmul noise).
The -0.5|e|^2 bias is preloaded into PSUM by the Scalar engine in exact
f32 and the matmuls accumulate onto it (start=False).  VectorE then
finds the row max (MAX8) and its first-match column (FIND_INDEX8)
directly on PSUM, and the quantize rows are gathered from the embed
table in DRAM with an indirect DMA.  x is pre-transposed on the host so
the contraction dim lands on SBUF partitions with no on-device
transposes.
"""

import os
import numpy as np

import concourse.bass as bass
import concourse.mybir as mybir
from concourse.tile import TileContext
from concourse.vector_clock import ScopedClock
from concourse.bass_utils import run_bass_kernel_spmd

N_CORES = 8
B, T, D = 16, 4096, 256
K = 2048
N_TOK = B * T                 # 65536
TOK_PER_CORE = N_TOK // N_CORES   # 8192
P = 128
N_TILES = TOK_PER_CORE // P   # 64
N_CHUNK = 4                   # 512-wide PSUM-bank chunks of the K axis
CHUNK = K // N_CHUNK
NEG_INF = -3.0e38

F32 = mybir.dt.float32
F32R = mybir.dt.float32r
I32 = mybir.dt.int32
U32 = mybir.dt.uint32


class _TileContextSplitDrain(TileContext):
    """This walrus build rejects CTRL instructions carrying more than ~1
    sync wait; split the exit-drain's waits across standalone NOPs."""

    def _drain_and_barrier(self, tick_clock, wait_clock):
        nc = self.nc
        placeholders = [nc.sync.nop(nofuse=True) for _ in range(48)]
        drain_inst = nc.sync.drain()
        wait_clock.add_sem_waits(
            drain_inst.ins, ScopedClock({None: tick_clock.global_clock})
        )
        si = drain_inst.ins.sync_info
        waits = list(si.on_wait) if si and si.on_wait else []
        if len(waits) > 1:
            keep, extra = waits[:1], waits[1:]
            assert len(extra) <= len(placeholders)
            for w, nop in zip(extra, placeholders):
                nop.ins.sync_info = mybir.SyncInfo(on_wait=[w], on_update=[])
            drain_inst.ins.sync_info = mybir.SyncInfo(
                on_wait=keep, on_update=list(si.on_update) if si.on_update else []
            )
        nc.all_engine_barrier()
        assert self.sems is not None
        popped = nc._tile_sem_poison_stack.pop()
        assert popped is self._sem_poison
        nc.clear_and_free_semaphores(list(self.sems.allocated().values()))
        nc.all_engine_barrier()


_ws_counter = [0]


def _split_sync_waits(nc, max_waits: int = 1):
    """This walrus build rejects instructions carrying more than ~1 sync
    wait; move excess waits onto same-engine NOPs inserted just before
    the carrying instruction (waits are AND conditions evaluated by the
    engine sequencer, so this is semantically equivalent)."""
    for func in nc.m.functions:
        for bb in func.blocks:
            insts = bb.instructions
            i = 0
            while i < len(insts):
                inst = insts[i]
                si = inst.sync_info
                if si is not None and si.on_wait and len(si.on_wait) > max_waits:
                    waits = list(si.on_wait)
                    keep = waits[-max_waits:]
                    extra = waits[:-max_waits]
                    nops = []
                    for j in range(0, len(extra), max_waits):
                        _ws_counter[0] += 1
                        nop = mybir.InstNoOp(name=f"I-waitsplit-{_ws_counter[0]}")
                        nop.engine = inst.engine
                        nop.sync_info = mybir.SyncInfo(
                            on_wait=extra[j : j + max_waits], on_update=[]
                        )
                        nops.append(nop)
                    inst.sync_info = mybir.SyncInfo(
                        on_wait=keep,
                        on_update=list(si.on_update) if si.on_update else [],
                    )
                    for k, nop in enumerate(nops):
                        insts.insert(i + k, nop)
                        nc.register_instruction(nop, overwrite=True)
                    i += len(nops)
                i += 1


def _build():
    nc = bass.Bass("TRN2")
    xth_d = nc.declare_dram_parameter("xth", [D, TOK_PER_CORE], F32R, isOutput=False)
    xtl_d = nc.declare_dram_parameter("xtl", [D, TOK_PER_CORE], F32R, isOutput=False)
    eth_d = nc.declare_dram_parameter("eth", [D, K], F32R, isOutput=False)
    etl_d = nc.declare_dram_parameter("etl", [D, K], F32R, isOutput=False)
    embed_d = nc.declare_dram_parameter("embed", [K, D], F32, isOutput=False)
    negb_d = nc.declare_dram_parameter("negb", [1, K], F32, isOutput=False)
    q_d = nc.declare_dram_parameter("q", [TOK_PER_CORE, D], F32, isOutput=True)
    ind_d = nc.declare_dram_parameter("ind", [P, N_TILES], I32, isOutput=True)

    with _TileContextSplitDrain(nc) as tc:
        with (
            tc.tile_pool(name="const", bufs=1) as const_pool,
            tc.tile_pool(name="xt", bufs=6) as xt_pool,
            tc.tile_pool(name="small", bufs=12) as small_pool,
            tc.tile_pool(name="q", bufs=6) as q_pool,
            tc.tile_pool(name="psum", bufs=4, space="PSUM") as psum_pool,
        ):
            # ---- constants, loaded once ----
            # split the embT loads per k-chunk so tile 0's first matmuls can
            # start as soon as their chunk lands
            eth_sb = const_pool.tile([P, 2, K], F32R)  # [d%128, d//128, k]
            etl_sb = const_pool.tile([P, 2, K], F32R)
            for kk in range(2):
                nc.sync.dma_start(
                    out=eth_sb[:, kk, :],
                    in_=eth_d.rearrange("(c p) k -> p c k", p=P)[:, kk, :],
                )
                nc.sync.dma_start(
                    out=etl_sb[:, kk, :],
                    in_=etl_d.rearrange("(c p) k -> p c k", p=P)[:, kk, :],
                )
            # bias row replicated across partitions (for the ACT preload)
            negb_sb = const_pool.tile([P, K], F32)
            nc.sync.dma_start(out=negb_sb, in_=negb_d[0:1, :].to_broadcast([P, K]))
            # persistent: output-index accumulator and the FIND match slots
            ind_all = const_pool.tile([P, N_TILES], I32)
            # zero row for the PSUM has_written init matmuls (memset can't
            # target f32r, so keep it f32 and bitcast at the matmul)
            zrow_f32 = const_pool.tile([1, CHUNK], F32)
            nc.vector.memset(zrow_f32, 0.0)
            zrow = zrow_f32.bitcast(F32R)

            for t in range(N_TILES):
                # ---- load xT tile [d, tok] hi/lo as 2 partition chunks ----
                # xt loads go through the scalar engine's HWDGE ring so the
                # q-store DMAs (which wait on each tile's gather) can't
                # head-of-line-block them on the sync ring
                xth_sb = xt_pool.tile([P, 2, P], F32R)
                xtl_sb = xt_pool.tile([P, 2, P], F32R)
                tok = slice(t * P, (t + 1) * P)
                nc.scalar.dma_start(
                    out=xth_sb,
                    in_=xth_d.rearrange("(c p) n -> p c n", p=P)[:, :, tok],
                )
                nc.scalar.dma_start(
                    out=xtl_sb,
                    in_=xtl_d.rearrange("(c p) n -> p c n", p=P)[:, :, tok],
                )

                # ---- per half-row (2 PSUM banks each, 4 in flight):
                #      exact-f32 bias preload, 3-pass f32r matmul,
                #      row max + first-match index directly on PSUM ----
                terms = [
                    (xth_sb, eth_sb),
                    (xth_sb, etl_sb),
                    (xtl_sb, eth_sb),
                ]
                HK = K // 2  # 1024 columns per half
                vmax8s, idx8s = [], []
                for h in range(2):
                    s_psum = psum_pool.tile([P, HK], F32)
                    if t < 2:
                        # PSUM has_written bits start in an unknown state; a
                        # start=True zero matmul per bank forces them SET so
                        # the start=False accumulation below lands on the
                        # ACT-written bias instead of overwriting it.  Bits
                        # persist across generations, so only the first
                        # bufs(=4) generations need this.
                        for c in range(2):
                            nc.tensor.matmul(
                                out=s_psum[:, c * CHUNK : (c + 1) * CHUNK],
                                lhsT=zrow[:, :P],
                                rhs=zrow,
                                start=True,
                                stop=False,
                                skip_group_check=True,
                            )
                    nc.scalar.copy(out=s_psum, in_=negb_sb[:, h * HK : (h + 1) * HK])
                    for k in range(2):
                        for ti, (xa, eb) in enumerate(terms):
                            last = k == 1 and ti == len(terms) - 1
                            for c in range(2):
                                kcol = h * HK + c * CHUNK
                                nc.tensor.matmul(
                                    out=s_psum[:, c * CHUNK : (c + 1) * CHUNK],
                                    lhsT=xa[:, k, :],
                                    rhs=eb[:, k, kcol : kcol + CHUNK],
                                    start=False,
                                    stop=last,
                                    skip_group_check=True,
                                )
                    vmax8 = small_pool.tile([P, 8], F32, tag="vmax8")
                    nc.vector.max(out=vmax8, in_=s_psum)
                    idx8 = small_pool.tile([P, 8], U32, tag="idx8")
                    nc.vector.max_index(out=idx8, in_max=vmax8, in_values=s_psum)
                    vmax8s.append(vmax8)
                    idx8s.append(idx8)

                # ---- combine halves: pick the half with the larger max ----
                ge = small_pool.tile([P, 1], U32, tag="ge")
                nc.vector.tensor_tensor(
                    out=ge,
                    in0=vmax8s[0][:, 0:1],
                    in1=vmax8s[1][:, 0:1],
                    op=mybir.AluOpType.is_ge,
                )
                idx1p = small_pool.tile([P, 1], U32, tag="idx1p")
                nc.vector.tensor_scalar_add(idx1p, idx8s[1][:, 0:1], HK)
                idx_sel = small_pool.tile([P, 1], U32, tag="idx_sel")
                nc.vector.select(
                    out=idx_sel, mask=ge, on_true=idx8s[0][:, 0:1], on_false=idx1p
                )

                # ---- record index (int32; on the lightly-loaded ACT) ----
                nc.scalar.copy(out=ind_all[:, t : t + 1], in_=idx_sel)

                # ---- gather embed rows -> quantize tile ----
                q_sb = q_pool.tile([P, D], F32)
                nc.gpsimd.indirect_dma_start(
                    out=q_sb,
                    out_offset=None,
                    in_=embed_d[:],
                    in_offset=bass.IndirectOffsetOnAxis(ap=idx_sel, axis=0),
                )
                nc.sync.dma_start(out=q_d[t * P : (t + 1) * P, :], in_=q_sb)

            nc.sync.dma_start(out=ind_d[:], in_=ind_all)
    _split_sync_waits(nc)
    return nc


_CACHE = {}


def _get_nc():
    if "nc" not in _CACHE:
        _CACHE["nc"] = _build()
    return _CACHE["nc"]


def _split_hi_lo(a: np.ndarray):
    """Split f32 array into hi + lo with hi exactly representable in the
    PE's fp32r format (11 explicit mantissa bits): truncate the low 12
    mantissa bits.  lo = a - hi is exact in f32 and has <= 12 significand
    bits, so fp32r loses at most its LSB (~2^-24 relative to a)."""
    hi = (a.view(np.uint32) & np.uint32(0xFFFFF000)).view(np.float32)
    lo = a - hi
    return hi, lo


def kernel(x: np.ndarray, embed: np.ndarray):
    x = np.ascontiguousarray(x, dtype=np.float32)
    embed = np.ascontiguousarray(embed, dtype=np.float32)
    flat = x.reshape(N_TOK, D)
    embt = np.ascontiguousarray(embed.T)
    eth, etl = _split_hi_lo(embt)
    negb = (-0.5 * np.sum(embed.astype(np.float64) ** 2, axis=1)).astype(
        np.float32
    )[None, :]

    in_maps = []
    for c in range(N_CORES):
        shard = flat[c * TOK_PER_CORE : (c + 1) * TOK_PER_CORE]
        xt = np.ascontiguousarray(shard.T)
        xth, xtl = _split_hi_lo(xt)
        in_maps.append(
            {
                "xth": xth,
                "xtl": xtl,
                "eth": eth,
                "etl": etl,
                "embed": embed,
                "negb": negb,
            }
        )

    nc = _get_nc()
    trace = bool(int(os.environ.get("VQ_TRACE", "0")))
    res = None
    last_err = None
    for attempt in range(3):
        try:
            res = run_bass_kernel_spmd(
                nc, in_maps, core_ids=list(range(N_CORES)), trace=trace
            )
            break
        except Exception as e:  # transient device wedges recover on retry
            last_err = e
            import time as _time

            _time.sleep(5)
    if res is None:
        raise last_err
    _CACHE["last_result"] = res

    quant = np.empty((N_TOK, D), dtype=np.float32)
    ind = np.empty(N_TOK, dtype=np.int32)
    for c in range(N_CORES):
        r = res.results[c]
        quant[c * TOK_PER_CORE : (c + 1) * TOK_PER_CORE] = r["q"]
        # ind layout on device: [p, tile] with token = tile*128 + p
        ind[c * TOK_PER_CORE : (c + 1) * TOK_PER_CORE] = np.ascontiguousarray(
            r["ind"].T
        ).reshape(-1)

    return quant.reshape(B, T, D), ind.reshape(B, T)


# revision 28
# speedup vs baseline: 1.0302x; 1.0302x over previous
"""EuclideanCodebook (VQ) kernel for 8 Trainium2 NeuronCores.

Computes, for x [16, 4096, 256] f32 and embed [2048, 256] f32:
    dist[n, k] = -(|x_n|^2 - 2 x_n.e_k + |e_k|^2)
    ind[n]     = argmax_k dist[n, k]
    quantize   = embed[ind]
returning (quantize [16,4096,256] f32, ind [16,4096] int32).

Strategy: data-parallel over the flattened token axis (65536 tokens ->
8192 per core).  argmax_k dist = argmax_k (x.e_k - 0.5|e_k|^2).

Exact-f32 scores at 3 cycles/row instead of the hardware fp32 matmul's
4: the PE's fp32r mode rounds inputs to 11 mantissa bits, so split
x = hi + lo and W = Whi + Wlo (host-side truncation at bit 11; every
term is fp32r-exact) and accumulate hi.Whi + hi.Wlo + lo.Whi in PSUM
(the dropped lo.Wlo term is ~2^-22 relative, below fp32 matmul noise).
The -0.5|e|^2 bias is preloaded into PSUM by the Scalar engine in exact
f32 and the matmuls accumulate onto it (start=False).  VectorE then
finds the row max (MAX8) and its first-match column (FIND_INDEX8)
directly on PSUM, and the quantize rows are gathered from the embed
table in DRAM with an indirect DMA.  x is pre-transposed on the host so
the contraction dim lands on SBUF partitions with no on-device
transposes.
"""

import os
import numpy as np

import concourse.bass as bass
import concourse.mybir as mybir
from concourse.tile import TileContext
from concourse.vector_clock import ScopedClock
from concourse.bass_utils import run_bass_kernel_spmd

N_CORES = 8
B, T, D = 16, 4096, 256
K = 2048
N_TOK = B * T                 # 65536
TOK_PER_CORE = N_TOK // N_CORES   # 8192
P = 128
N_TILES = TOK_PER_CORE // P   # 64
N_CHUNK = 4                   # 512-wide PSUM-bank chunks of the K axis
CHUNK = K // N_CHUNK
NEG_INF = -3.0e38

F32 = mybir.dt.float32
F32R = mybir.dt.float32r
I32 = mybir.dt.int32
U32 = mybir.dt.uint32


class _TileContextSplitDrain(TileContext):
    """This walrus build rejects CTRL instructions carrying more than ~1
    sync wait; split the exit-drain's waits across standalone NOPs."""

    def _drain_and_barrier(self, tick_clock, wait_clock):
        nc = self.nc
        placeholders = [nc.sync.nop(nofuse=True) for _ in range(48)]
        drain_inst = nc.sync.drain()
        wait_clock.add_sem_waits(
            drain_inst.ins, ScopedClock({None: tick_clock.global_clock})
        )
        si = drain_inst.ins.sync_info
        waits = list(si.on_wait) if si and si.on_wait else []
        if len(waits) > 1:
            keep, extra = waits[:1], waits[1:]
            assert len(extra) <= len(placeholders)
            for w, nop in zip(extra, placeholders):
                nop.ins.sync_info = mybir.SyncInfo(on_wait=[w], on_update=[])
            drain_inst.ins.sync_info = mybir.SyncInfo(
                on_wait=keep, on_update=list(si.on_update) if si.on_update else []
            )
        nc.all_engine_barrier()
        assert self.sems is not None
        popped = nc._tile_sem_poison_stack.pop()
        assert popped is self._sem_poison
        nc.clear_and_free_semaphores(list(self.sems.allocated().values()))
        nc.all_engine_barrier()


_ws_counter = [0]


def _split_sync_waits(nc, max_waits: int = 1):
    """This walrus build rejects instructions carrying more than ~1 sync
    wait; move excess waits onto same-engine NOPs inserted just before
    the carrying instruction (waits are AND conditions evaluated by the
    engine sequencer, so this is semantically equivalent)."""
    for func in nc.m.functions:
        for bb in func.blocks:
            insts = bb.instructions
            i = 0
            while i < len(insts):
                inst = insts[i]
                si = inst.sync_info
                if si is not None and si.on_wait and len(si.on_wait) > max_waits:
                    waits = list(si.on_wait)
                    keep = waits[-max_waits:]
                    extra = waits[:-max_waits]
                    nops = []
                    for j in range(0, len(extra), max_waits):
                        _ws_counter[0] += 1
                        nop = mybir.InstNoOp(name=f"I-waitsplit-{_ws_counter[0]}")
                        nop.engine = inst.engine
                        nop.sync_info = mybir.SyncInfo(
                            on_wait=extra[j : j + max_waits], on_update=[]
                        )
                        nops.append(nop)
                    inst.sync_info = mybir.SyncInfo(
                        on_wait=keep,
                        on_update=list(si.on_update) if si.on_update else [],
                    )
                    for k, nop in enumerate(nops):
                        insts.insert(i + k, nop)
                        nc.register_instruction(nop, overwrite=True)
                    i += len(nops)
                i += 1


def _build():
    nc = bass.Bass("TRN2")
    xth_d = nc.declare_dram_parameter("xth", [D, TOK_PER_CORE], F32R, isOutput=False)
    xtl_d = nc.declare_dram_parameter("xtl", [D, TOK_PER_CORE], F32R, isOutput=False)
    eth_d = nc.declare_dram_parameter("eth", [D, K], F32R, isOutput=False)
    etl_d = nc.declare_dram_parameter("etl", [D, K], F32R, isOutput=False)
    embed_d = nc.declare_dram_parameter("embed", [K, D], F32, isOutput=False)
    negb_d = nc.declare_dram_parameter("negb", [1, K], F32, isOutput=False)
    q_d = nc.declare_dram_parameter("q", [TOK_PER_CORE, D], F32, isOutput=True)
    ind_d = nc.declare_dram_parameter("ind", [P, N_TILES], I32, isOutput=True)

    with _TileContextSplitDrain(nc) as tc:
        with (
            tc.tile_pool(name="const", bufs=1) as const_pool,
            tc.tile_pool(name="xt", bufs=6) as xt_pool,
            tc.tile_pool(name="small", bufs=12) as small_pool,
            tc.tile_pool(name="q", bufs=6) as q_pool,
            tc.tile_pool(name="psum", bufs=4, space="PSUM") as psum_pool,
        ):
            # ---- constants ----
            # Startup critical path: tile 0 needs negb (bias preload), the
            # k=0 chunk of eth, and its own x tile before the first matmul.
            # Issue those first on the FIFO DMA queue; the remaining embT
            # chunks are issued inside the t==0 body after tile 0/1's loads.
            eth_sb = const_pool.tile([P, 2, K], F32R)  # [d%128, d//128, k]
            etl_sb = const_pool.tile([P, 2, K], F32R)
            negb_sb = const_pool.tile([P, K], F32)
            nc.sync.dma_start(out=negb_sb, in_=negb_d[0:1, :].to_broadcast([P, K]))
            nc.sync.dma_start(
                out=eth_sb[:, 0, :],
                in_=eth_d.rearrange("(c p) k -> p c k", p=P)[:, 0, :],
            )

            def _load_late_consts():
                nc.sync.dma_start(
                    out=etl_sb[:, 0, :],
                    in_=etl_d.rearrange("(c p) k -> p c k", p=P)[:, 0, :],
                )
                for kk in (1,):
                    nc.sync.dma_start(
                        out=eth_sb[:, kk, :],
                        in_=eth_d.rearrange("(c p) k -> p c k", p=P)[:, kk, :],
                    )
                    nc.sync.dma_start(
                        out=etl_sb[:, kk, :],
                        in_=etl_d.rearrange("(c p) k -> p c k", p=P)[:, kk, :],
                    )
            # persistent: output-index accumulator and the FIND match slots
            ind_all = const_pool.tile([P, N_TILES], I32)
            # zero row for the PSUM has_written init matmuls (memset can't
            # target f32r, so keep it f32 and bitcast at the matmul)
            zrow_f32 = const_pool.tile([1, CHUNK], F32)
            nc.vector.memset(zrow_f32, 0.0)
            zrow = zrow_f32.bitcast(F32R)

            for t in range(N_TILES):
                # ---- load xT tile [d, tok] hi/lo as 2 partition chunks ----
                xth_sb = xt_pool.tile([P, 2, P], F32R)
                xtl_sb = xt_pool.tile([P, 2, P], F32R)
                tok = slice(t * P, (t + 1) * P)
                nc.sync.dma_start(
                    out=xth_sb,
                    in_=xth_d.rearrange("(c p) n -> p c n", p=P)[:, :, tok],
                )
                nc.sync.dma_start(
                    out=xtl_sb,
                    in_=xtl_d.rearrange("(c p) n -> p c n", p=P)[:, :, tok],
                )
                if t == 0:
                    _load_late_consts()

                # ---- per half-row (2 PSUM banks each, 4 in flight):
                #      exact-f32 bias preload, 3-pass f32r matmul,
                #      row max + first-match index directly on PSUM ----
                # (hi,eth) and (lo,eth) first: tile 0's k=0 matmuls only
                # need the eth chunk that was issued before the loop
                terms = [
                    (xth_sb, eth_sb),
                    (xtl_sb, eth_sb),
                    (xth_sb, etl_sb),
                ]
                HK = K // 2  # 1024 columns per half
                vmax8s, idx8s = [], []
                for h in range(2):
                    s_psum = psum_pool.tile([P, HK], F32)
                    if t < 2:
                        # PSUM has_written bits start in an unknown state; a
                        # start=True zero matmul per bank forces them SET so
                        # the start=False accumulation below lands on the
                        # ACT-written bias instead of overwriting it.  Bits
                        # persist across generations, so only the first
                        # bufs(=4) generations need this.
                        for c in range(2):
                            nc.tensor.matmul(
                                out=s_psum[:, c * CHUNK : (c + 1) * CHUNK],
                                lhsT=zrow[:, :P],
                                rhs=zrow,
                                start=True,
                                stop=False,
                                skip_group_check=True,
                            )
                    nc.scalar.copy(out=s_psum, in_=negb_sb[:, h * HK : (h + 1) * HK])
                    for k in range(2):
                        for ti, (xa, eb) in enumerate(terms):
                            last = k == 1 and ti == len(terms) - 1
                            for c in range(2):
                                kcol = h * HK + c * CHUNK
                                nc.tensor.matmul(
                                    out=s_psum[:, c * CHUNK : (c + 1) * CHUNK],
                                    lhsT=xa[:, k, :],
                                    rhs=eb[:, k, kcol : kcol + CHUNK],
                                    start=False,
                                    stop=last,
                                    skip_group_check=True,
                                )
                    vmax8 = small_pool.tile([P, 8], F32, tag="vmax8")
                    nc.vector.max(out=vmax8, in_=s_psum)
                    idx8 = small_pool.tile([P, 8], U32, tag="idx8")
                    nc.vector.max_index(out=idx8, in_max=vmax8, in_values=s_psum)
                    vmax8s.append(vmax8)
                    idx8s.append(idx8)

                # ---- combine halves: pick the half with the larger max ----
                ge = small_pool.tile([P, 1], U32, tag="ge")
                nc.vector.tensor_tensor(
                    out=ge,
                    in0=vmax8s[0][:, 0:1],
                    in1=vmax8s[1][:, 0:1],
                    op=mybir.AluOpType.is_ge,
                )
                idx1p = small_pool.tile([P, 1], U32, tag="idx1p")
                nc.vector.tensor_scalar_add(idx1p, idx8s[1][:, 0:1], HK)
                idx_sel = small_pool.tile([P, 1], U32, tag="idx_sel")
                nc.vector.select(
                    out=idx_sel, mask=ge, on_true=idx8s[0][:, 0:1], on_false=idx1p
                )

                # ---- record index (int32; on the lightly-loaded ACT) ----
                nc.scalar.copy(out=ind_all[:, t : t + 1], in_=idx_sel)

                # ---- gather embed rows -> quantize tile ----
                q_sb = q_pool.tile([P, D], F32)
                nc.gpsimd.indirect_dma_start(
                    out=q_sb,
                    out_offset=None,
                    in_=embed_d[:],
                    in_offset=bass.IndirectOffsetOnAxis(ap=idx_sel, axis=0),
                )
                nc.sync.dma_start(out=q_d[t * P : (t + 1) * P, :], in_=q_sb)

            nc.sync.dma_start(out=ind_d[:], in_=ind_all)
    _split_sync_waits(nc)
    return nc


_CACHE = {}


def _get_nc():
    if "nc" not in _CACHE:
        _CACHE["nc"] = _build()
    return _CACHE["nc"]


def _split_hi_lo(a: np.ndarray):
    """Split f32 array into hi + lo with hi exactly representable in the
    PE's fp32r format (11 explicit mantissa bits): truncate the low 12
    mantissa bits.  lo = a - hi is exact in f32 and has <= 12 significand
    bits, so fp32r loses at most its LSB (~2^-24 relative to a)."""
    hi = (a.view(np.uint32) & np.uint32(0xFFFFF000)).view(np.float32)
    lo = a - hi
    return hi, lo


def kernel(x: np.ndarray, embed: np.ndarray):
    x = np.ascontiguousarray(x, dtype=np.float32)
    embed = np.ascontiguousarray(embed, dtype=np.float32)
    flat = x.reshape(N_TOK, D)
    embt = np.ascontiguousarray(embed.T)
    eth, etl = _split_hi_lo(embt)
    negb = (-0.5 * np.sum(embed.astype(np.float64) ** 2, axis=1)).astype(
        np.float32
    )[None, :]

    in_maps = []
    for c in range(N_CORES):
        shard = flat[c * TOK_PER_CORE : (c + 1) * TOK_PER_CORE]
        xt = np.ascontiguousarray(shard.T)
        xth, xtl = _split_hi_lo(xt)
        in_maps.append(
            {
                "xth": xth,
                "xtl": xtl,
                "eth": eth,
                "etl": etl,
                "embed": embed,
                "negb": negb,
            }
        )

    nc = _get_nc()
    trace = bool(int(os.environ.get("VQ_TRACE", "0")))
    res = None
    last_err = None
    for attempt in range(3):
        try:
            res = run_bass_kernel_spmd(
                nc, in_maps, core_ids=list(range(N_CORES)), trace=trace
            )
            break
        except Exception as e:  # transient device wedges recover on retry
            last_err = e
            import time as _time

            _time.sleep(5)
    if res is None:
        raise last_err
    _CACHE["last_result"] = res

    quant = np.empty((N_TOK, D), dtype=np.float32)
    ind = np.empty(N_TOK, dtype=np.int32)
    for c in range(N_CORES):
        r = res.results[c]
        quant[c * TOK_PER_CORE : (c + 1) * TOK_PER_CORE] = r["q"]
        # ind layout on device: [p, tile] with token = tile*128 + p
        ind[c * TOK_PER_CORE : (c + 1) * TOK_PER_CORE] = np.ascontiguousarray(
            r["ind"].T
        ).reshape(-1)

    return quant.reshape(B, T, D), ind.reshape(B, T)
